# revision 17
# baseline (speedup 1.0000x reference)
"""Trainium2 Bass kernel for nn_DecSwitchedFC (MoE hard routing).

Math (per token b, expert e = y_idx[b]):
    out[b] = x[b] + z[b, e] * (relu(x[b] @ W1[e] + b1[e]) @ W2[e] + b2[e])

Strategy: expert-parallel over 8 NeuronCores, 2 experts per core.  The host
routes tokens to their experts (pure data movement — gather/scatter and
transpose), each core runs the two dense bottleneck FCs for its experts'
tokens on the tensor engine, applies bias/relu/route-scale/residual on the
scalar/vector engines, and the host scatters rows back.  Only the selected
expert is computed per token (1/16 of the reference FLOPs).

Device data layout is fully "feature-major" (d or h on partitions, tokens on
the free axis) so no on-device transposes are needed:
    h^T[256, n]  = W1[e]^T(lhsT=W1) @ x^T          (K=1024, 8 chunks)
    o^T[1024, n] = W2[e]^T(lhsT=W2) @ relu(h^T+b1) (K=256, 2 chunks)
    out^T        = (o^T + b2) * z + x^T

All host-side arrays are packed so every DMA is a single issue with 128
descriptors of contiguous bytes (one per SBUF partition) — DMA issue rate,
not bandwidth, limits the pipeline ramp otherwise.
"""

import numpy as np

import concourse.bacc as bacc
import concourse.mybir as mybir
import concourse.tile as tile
from concourse.bass_utils import run_bass_kernel_spmd

D = 1024        # model dim
H = 256         # bottleneck dim
NB = 16         # n experts
NCORES = 8
EPC = NB // NCORES   # experts per core
TILE_N = 512    # token-tile width (fp32 moving-operand max)
KC1 = D // 128  # contraction chunks for x @ W1
KC2 = H // 128  # contraction chunks for h @ W2
F32 = mybir.dt.float32
# Matmul operand dtype: float32 (exact, 4 cyc/row) or float32r
# (TF32-like ~1e-4 relative error, 1 cyc/row).
MM_DT = mybir.dt.float32r

_build_cache: dict[tuple, object] = {}
LAST_RESULTS = None  # BassKernelResults of the most recent run (for profiling)


def _tile_seq(caps):
    """Global tile order: 128-wide tile first (fast PE start via a small
    first DMA) and 128-wide tile last (short epilogue/drain tail).
    Returns [(s, t0, tn), ...]."""
    def chunks(cap, lead128, tail128):
        widths = []
        rem = cap
        if lead128 and rem > 128:
            widths.append(128)
            rem -= 128
        tail = 0
        if tail128 and rem > 128:
            tail = 128
            rem -= 128
        while rem > 0:
            w = min(TILE_N, rem)
            widths.append(w)
            rem -= w
        if tail:
            widths.append(tail)
        out = []
        t0 = 0
        for w in widths:
            out.append((t0, w))
            t0 += w
        return out
    seq = [(0, t0, tn) for t0, tn in chunks(caps[0], True, False)]
    seq += [(1, t0, tn) for t0, tn in chunks(caps[1], False, True)]
    return seq


def _build(caps):
    key = (caps, MM_DT)
    if key in _build_cache:
        return _build_cache[key]
    C = sum(caps)
    seq = _tile_seq(caps)
    xcols = KC1 * C   # total packed x / out columns

    nc = bacc.Bacc("TRN2", target_bir_lowering=False, debug=False)

    xg = nc.dram_tensor("xg", [128, xcols], MM_DT, kind="ExternalInput")
    zg = nc.dram_tensor("zg", [128, C], F32, kind="ExternalInput")
    w1 = nc.dram_tensor("w1", [128, EPC, KC1 * H], MM_DT, kind="ExternalInput")
    w2 = nc.dram_tensor("w2", [128, EPC, KC2 * D], MM_DT, kind="ExternalInput")
    # bias[p, e*10 + j]     = b1[e, 128j + p]   (j in 0..1)
    # bias[p, e*10 + 2 + i] = b2[e, 128i + p]   (i in 0..7)
    bias = nc.dram_tensor("bias", [128, EPC * (KC2 + KC1)], F32,
                          kind="ExternalInput")
    # packed out: per tile block of KC1*tn columns, [p, i*tn + c]
    outP = nc.dram_tensor("outP", [128, xcols], F32, kind="ExternalOutput")

    with tile.TileContext(nc) as tc:
        with (
            tc.tile_pool(name="const", bufs=1) as cpool,
            tc.tile_pool(name="w1p", bufs=2) as w1pool,
            tc.tile_pool(name="w2p", bufs=2) as w2pool,
            tc.tile_pool(name="xp", bufs=3) as xpool,
            tc.tile_pool(name="zp", bufs=3) as zpool,
            tc.tile_pool(name="hp", bufs=2) as hpool,
            tc.tile_pool(name="op", bufs=2) as opool,
            tc.tile_pool(name="ph", bufs=2, space="PSUM") as phpool,
            tc.tile_pool(name="po", bufs=4, space="PSUM") as popool,
        ):
            bias_t = None
            w1t = w2t = None
            cur_s = -1
            xoff = 0
            for q, (s, t0, tn) in enumerate(seq):
                new_s = s != cur_s
                cur_s = s
                if new_s:
                    w1t = w1pool.tile([128, KC1, H], MM_DT, tag="w1t")
                    nc.sync.dma_start(w1t[:, 0, :], w1[:, s, 0:H])

                xt = xpool.tile([128, KC1, tn], MM_DT, tag="xt")
                nc.sync.dma_start(
                    xt[:],
                    xg[:, xoff:xoff + KC1 * tn].rearrange(
                        "p (k c) -> p k c", k=KC1))
                if new_s:
                    for k in range(1, KC1):
                        nc.sync.dma_start(w1t[:, k, :],
                                          w1[:, s, k * H:(k + 1) * H])
                c0 = (caps[0] if s else 0) + t0
                zt = zpool.tile([128, tn], F32, tag="zt")
                nc.sync.dma_start(zt[:], zg[:, c0:c0 + tn])
                if bias_t is None:
                    bias_t = cpool.tile([128, EPC * (KC2 + KC1)], F32)
                    nc.sync.dma_start(bias_t[:], bias[:])
                if new_s:
                    w2t = w2pool.tile([128, KC2, D], MM_DT, tag="w2t")
                    nc.sync.dma_start(
                        w2t[:], w2[:, s].rearrange("p (j m) -> p j m", j=KC2))

                ht = hpool.tile([128, KC2, tn], MM_DT, tag="ht")
                for j in range(KC2):
                    ph = phpool.tile([128, tn], F32, tag="ph")
                    for k in range(KC1):
                        nc.tensor.matmul(
                            ph[:], w1t[:, k, 128 * j:128 * (j + 1)],
                            xt[:, k, :],
                            start=(k == 0), stop=(k == KC1 - 1))
                    nc.scalar.activation(
                        ht[:, j, :], ph[:],
                        mybir.ActivationFunctionType.Relu,
                        bias=bias_t[:, s * 10 + j:s * 10 + j + 1])

                ot = opool.tile([128, KC1, tn], F32, tag="ot")
                for i in range(KC1):
                    po = popool.tile([128, tn], F32, tag="po")
                    for j in range(KC2):
                        nc.tensor.matmul(
                            po[:], w2t[:, j, 128 * i:128 * (i + 1)],
                            ht[:, j, :],
                            start=(j == 0), stop=(j == KC2 - 1))
                    # (o + b2) * z
                    nc.vector.scalar_tensor_tensor(
                        ot[:, i, :], po[:],
                        bias_t[:, s * 10 + 2 + i:s * 10 + 3 + i],
                        zt[:],
                        mybir.AluOpType.add, mybir.AluOpType.mult)
                    # + x (residual)
                    nc.vector.tensor_add(ot[:, i, :], ot[:, i, :],
                                         xt[:, i, :].bitcast(F32))
                    if q == len(seq) - 1:
                        nc.sync.dma_start(
                            outP[:, xoff + i * tn:xoff + (i + 1) * tn],
                            ot[:, i, :])
                if q != len(seq) - 1:
                    nc.sync.dma_start(
                        outP[:, xoff:xoff + KC1 * tn].rearrange(
                            "p (k c) -> p k c", k=KC1),
                        ot[:])

                xoff += KC1 * tn

    nc.compile()
    _build_cache[key] = nc
    return nc


def kernel(x, y_idx, y, z, W1, b1, W2, b2):
    x = np.ascontiguousarray(np.asarray(x, dtype=np.float32))
    z = np.asarray(z, dtype=np.float32)
    W1 = np.asarray(W1, dtype=np.float32)
    b1 = np.asarray(b1, dtype=np.float32)
    W2 = np.asarray(W2, dtype=np.float32)
    b2 = np.asarray(b2, dtype=np.float32)
    e = np.asarray(y_idx).reshape(-1).astype(np.int64)
    B = x.shape[0]

    idxs = [np.flatnonzero(e == k) for k in range(NB)]
    counts = np.array([len(i) for i in idxs])
    # Assign experts to (core, slot): top-8 counts in slot 0, bottom-8 in
    # slot 1, so the per-slot padded capacities are as small as possible.
    order = np.argsort(-counts, kind="stable")
    assign = [[int(order[c]), int(order[NB - 1 - c])] for c in range(NCORES)]

    def _cap(ns):
        return max(128, -(-max(ns) // 128) * 128)

    caps = (_cap([counts[assign[c][0]] for c in range(NCORES)]),
            _cap([counts[assign[c][1]] for c in range(NCORES)]))
    C = sum(caps)
    seq = _tile_seq(caps)
    xcols = KC1 * C

    nc = _build(caps)

    nbias = EPC * (KC2 + KC1)
    in_maps = []
    for c in range(NCORES):
        xg = np.zeros((128, xcols), np.float32)
        zg = np.zeros((128, C), np.float32)
        bias = np.zeros((128, nbias), np.float32)
        w1 = np.empty((128, EPC, KC1 * H), np.float32)
        w2 = np.empty((128, EPC, KC2 * D), np.float32)
        for s in range(EPC):
            k = assign[c][s]
            idx = idxs[k]
            n = len(idx)
            c0 = caps[0] if s else 0
            zg[:, c0:c0 + n] = z[idx, k][None, :]
            bias[:, s * 10:s * 10 + KC2] = b1[k].reshape(KC2, 128).T
            bias[:, s * 10 + KC2:s * 10 + KC2 + KC1] = b2[k].reshape(KC1, 128).T
            w1[:, s] = W1[k].reshape(KC1, 128, H).transpose(1, 0, 2).reshape(
                128, KC1 * H)
            w2[:, s] = W2[k].reshape(KC2, 128, D).transpose(1, 0, 2).reshape(
                128, KC2 * D)
        xoff = 0
        for s, t0, tn in seq:
            k = assign[c][s]
            seg = idxs[k][t0:t0 + tn]
            n = len(seg)
            if n:
                full = np.zeros((128, KC1, tn), np.float32)
                full[:, :, :n] = x[seg].reshape(n, KC1, 128).transpose(2, 1, 0)
                xg[:, xoff:xoff + KC1 * tn] = full.reshape(128, KC1 * tn)
            xoff += KC1 * tn
        in_maps.append({"xg": xg, "zg": zg, "w1": w1, "w2": w2, "bias": bias})

    res = run_bass_kernel_spmd(nc, in_maps, core_ids=list(range(NCORES)))
    global LAST_RESULTS
    LAST_RESULTS = res

    out = np.empty((B, D), np.float32)
    for c in range(NCORES):
        outP = res.results[c]["outP"]
        xoff = 0
        for s, t0, tn in seq:
            k = assign[c][s]
            seg = idxs[k][t0:t0 + tn]
            n = len(seg)
            if n:
                blk = outP[:, xoff:xoff + KC1 * tn].reshape(128, KC1, tn)
                # blk[p, i, c] = out[token c, 128i + p]
                out[seg] = blk[:, :, :n].transpose(2, 1, 0).reshape(n, D)
            xoff += KC1 * tn
    return out


# revision 18
# speedup vs baseline: 1.0661x; 1.0661x over previous
"""Trainium2 Bass kernel for nn_DecSwitchedFC (MoE hard routing).

Math (per token b, expert e = y_idx[b]):
    out[b] = x[b] + z[b, e] * (relu(x[b] @ W1[e] + b1[e]) @ W2[e] + b2[e])

Strategy: expert-parallel over 8 NeuronCores, 2 experts per core.  The host
routes tokens to their experts (pure data movement — gather/scatter and
transpose), each core runs the two dense bottleneck FCs for its experts'
tokens on the tensor engine, applies bias/relu/route-scale/residual on the
scalar/vector engines, and the host scatters rows back.  Only the selected
expert is computed per token (1/16 of the reference FLOPs).

Device data layout is fully "feature-major" (d or h on partitions, tokens on
the free axis) so no on-device transposes are needed:
    h^T[256, n]  = W1[e]^T(lhsT=W1) @ x^T          (K=1024, 8 chunks)
    o^T[1024, n] = W2[e]^T(lhsT=W2) @ relu(h^T+b1) (K=256, 2 chunks)
    out^T        = (o^T + b2) * z + x^T

All host-side arrays are packed so every DMA is a single issue with 128
descriptors of contiguous bytes (one per SBUF partition) — DMA issue rate,
not bandwidth, limits the pipeline ramp otherwise.
"""

import numpy as np

import concourse.bacc as bacc
import concourse.mybir as mybir
import concourse.tile as tile
from concourse.bass_utils import run_bass_kernel_spmd

D = 1024        # model dim
H = 256         # bottleneck dim
NB = 16         # n experts
NCORES = 8
EPC = NB // NCORES   # experts per core
TILE_N = 512    # token-tile width (fp32 moving-operand max)
KC1 = D // 128  # contraction chunks for x @ W1
KC2 = H // 128  # contraction chunks for h @ W2
F32 = mybir.dt.float32
# Matmul operand dtype: float32 (exact, 4 cyc/row) or float32r
# (TF32-like ~1e-4 relative error, 1 cyc/row).
MM_DT = mybir.dt.float32r

_build_cache: dict[tuple, object] = {}
LAST_RESULTS = None  # BassKernelResults of the most recent run (for profiling)


def _tile_seq(caps):
    """Global tile order: 128-wide tile first (fast PE start via a small
    first DMA) and 128-wide tile last (short epilogue/drain tail).
    Returns [(s, t0, tn), ...]."""
    def chunks(cap, lead128, tail128):
        widths = []
        rem = cap
        if lead128 and rem > 128:
            widths.append(128)
            rem -= 128
        tail = 0
        if tail128 and rem > 128:
            tail = 128
            rem -= 128
        while rem > 0:
            w = min(TILE_N, rem)
            widths.append(w)
            rem -= w
        if tail:
            widths.append(tail)
        out = []
        t0 = 0
        for w in widths:
            out.append((t0, w))
            t0 += w
        return out
    seq = [(0, t0, tn) for t0, tn in chunks(caps[0], True, False)]
    seq += [(1, t0, tn) for t0, tn in chunks(caps[1], False, True)]
    return seq


def _build(caps):
    key = (caps, MM_DT)
    if key in _build_cache:
        return _build_cache[key]
    C = sum(caps)
    seq = _tile_seq(caps)
    xcols = KC1 * C   # total packed x / out columns

    nc = bacc.Bacc("TRN2", target_bir_lowering=False, debug=False)

    xg = nc.dram_tensor("xg", [128, xcols], MM_DT, kind="ExternalInput")
    zg = nc.dram_tensor("zg", [128, C], F32, kind="ExternalInput")
    w1 = nc.dram_tensor("w1", [128, EPC, KC1 * H], MM_DT, kind="ExternalInput")
    w2 = nc.dram_tensor("w2", [128, EPC, KC2 * D], MM_DT, kind="ExternalInput")
    # bias[p, e*10 + j]     = b1[e, 128j + p]   (j in 0..1)
    # bias[p, e*10 + 2 + i] = b2[e, 128i + p]   (i in 0..7)
    bias = nc.dram_tensor("bias", [128, EPC * (KC2 + KC1)], F32,
                          kind="ExternalInput")
    # packed out: per tile block of KC1*tn columns, [p, i*tn + c]
    outP = nc.dram_tensor("outP", [128, xcols], F32, kind="ExternalOutput")

    with tile.TileContext(nc) as tc:
        with (
            tc.tile_pool(name="const", bufs=1) as cpool,
            tc.tile_pool(name="w1p", bufs=2) as w1pool,
            tc.tile_pool(name="w2p", bufs=2) as w2pool,
            tc.tile_pool(name="xp", bufs=3) as xpool,
            tc.tile_pool(name="zp", bufs=3) as zpool,
            tc.tile_pool(name="hp", bufs=2) as hpool,
            tc.tile_pool(name="op", bufs=2) as opool,
            tc.tile_pool(name="ph", bufs=2, space="PSUM") as phpool,
            tc.tile_pool(name="po", bufs=4, space="PSUM") as popool,
        ):
            bias_t = None
            w1t = w2t = None
            cur_s = -1
            xoff = 0
            for q, (s, t0, tn) in enumerate(seq):
                new_s = s != cur_s
                cur_s = s
                if new_s:
                    w1t = w1pool.tile([128, KC1, H], MM_DT, tag="w1t")
                    nc.sync.dma_start(
                        w1t[:], w1[:, s].rearrange("p (k m) -> p k m", k=KC1))

                xt = xpool.tile([128, KC1, tn], MM_DT, tag="xt")
                nc.sync.dma_start(
                    xt[:],
                    xg[:, xoff:xoff + KC1 * tn].rearrange(
                        "p (k c) -> p k c", k=KC1))
                c0 = (caps[0] if s else 0) + t0
                zt = zpool.tile([128, tn], F32, tag="zt")
                nc.sync.dma_start(zt[:], zg[:, c0:c0 + tn])
                if bias_t is None:
                    bias_t = cpool.tile([128, EPC * (KC2 + KC1)], F32)
                    nc.sync.dma_start(bias_t[:], bias[:])
                if new_s:
                    w2t = w2pool.tile([128, KC2, D], MM_DT, tag="w2t")
                    nc.sync.dma_start(
                        w2t[:], w2[:, s].rearrange("p (j m) -> p j m", j=KC2))

                ht = hpool.tile([128, KC2, tn], MM_DT, tag="ht")
                for j in range(KC2):
                    ph = phpool.tile([128, tn], F32, tag="ph")
                    for k in range(KC1):
                        nc.tensor.matmul(
                            ph[:], w1t[:, k, 128 * j:128 * (j + 1)],
                            xt[:, k, :],
                            start=(k == 0), stop=(k == KC1 - 1))
                    nc.scalar.activation(
                        ht[:, j, :], ph[:],
                        mybir.ActivationFunctionType.Relu,
                        bias=bias_t[:, s * 10 + j:s * 10 + j + 1])

                ot = opool.tile([128, KC1, tn], F32, tag="ot")
                for i in range(KC1):
                    po = popool.tile([128, tn], F32, tag="po")
                    for j in range(KC2):
                        nc.tensor.matmul(
                            po[:], w2t[:, j, 128 * i:128 * (i + 1)],
                            ht[:, j, :],
                            start=(j == 0), stop=(j == KC2 - 1))
                    # (o + b2) * z
                    nc.vector.scalar_tensor_tensor(
                        ot[:, i, :], po[:],
                        bias_t[:, s * 10 + 2 + i:s * 10 + 3 + i],
                        zt[:],
                        mybir.AluOpType.add, mybir.AluOpType.mult)
                    # + x (residual)
                    nc.vector.tensor_add(ot[:, i, :], ot[:, i, :],
                                         xt[:, i, :].bitcast(F32))
                    if q == len(seq) - 1:
                        nc.sync.dma_start(
                            outP[:, xoff + i * tn:xoff + (i + 1) * tn],
                            ot[:, i, :])
                if q != len(seq) - 1:
                    nc.sync.dma_start(
                        outP[:, xoff:xoff + KC1 * tn].rearrange(
                            "p (k c) -> p k c", k=KC1),
                        ot[:])

                xoff += KC1 * tn

    nc.compile()
    _build_cache[key] = nc
    return nc


def kernel(x, y_idx, y, z, W1, b1, W2, b2):
    x = np.ascontiguousarray(np.asarray(x, dtype=np.float32))
    z = np.asarray(z, dtype=np.float32)
    W1 = np.asarray(W1, dtype=np.float32)
    b1 = np.asarray(b1, dtype=np.float32)
    W2 = np.asarray(W2, dtype=np.float32)
    b2 = np.asarray(b2, dtype=np.float32)
    e = np.asarray(y_idx).reshape(-1).astype(np.int64)
    B = x.shape[0]

    idxs = [np.flatnonzero(e == k) for k in range(NB)]
    counts = np.array([len(i) for i in idxs])
    # Assign experts to (core, slot): top-8 counts in slot 0, bottom-8 in
    # slot 1, so the per-slot padded capacities are as small as possible.
    order = np.argsort(-counts, kind="stable")
    assign = [[int(order[c]), int(order[NB - 1 - c])] for c in range(NCORES)]

    def _cap(ns):
        return max(128, -(-max(ns) // 128) * 128)

    caps = (_cap([counts[assign[c][0]] for c in range(NCORES)]),
            _cap([counts[assign[c][1]] for c in range(NCORES)]))
    C = sum(caps)
    seq = _tile_seq(caps)
    xcols = KC1 * C

    nc = _build(caps)

    nbias = EPC * (KC2 + KC1)
    in_maps = []
    for c in range(NCORES):
        xg = np.zeros((128, xcols), np.float32)
        zg = np.zeros((128, C), np.float32)
        bias = np.zeros((128, nbias), np.float32)
        w1 = np.empty((128, EPC, KC1 * H), np.float32)
        w2 = np.empty((128, EPC, KC2 * D), np.float32)
        for s in range(EPC):
            k = assign[c][s]
            idx = idxs[k]
            n = len(idx)
            c0 = caps[0] if s else 0
            zg[:, c0:c0 + n] = z[idx, k][None, :]
            bias[:, s * 10:s * 10 + KC2] = b1[k].reshape(KC2, 128).T
            bias[:, s * 10 + KC2:s * 10 + KC2 + KC1] = b2[k].reshape(KC1, 128).T
            w1[:, s] = W1[k].reshape(KC1, 128, H).transpose(1, 0, 2).reshape(
                128, KC1 * H)
            w2[:, s] = W2[k].reshape(KC2, 128, D).transpose(1, 0, 2).reshape(
                128, KC2 * D)
        xoff = 0
        for s, t0, tn in seq:
            k = assign[c][s]
            seg = idxs[k][t0:t0 + tn]
            n = len(seg)
            if n:
                full = np.zeros((128, KC1, tn), np.float32)
                full[:, :, :n] = x[seg].reshape(n, KC1, 128).transpose(2, 1, 0)
                xg[:, xoff:xoff + KC1 * tn] = full.reshape(128, KC1 * tn)
            xoff += KC1 * tn
        in_maps.append({"xg": xg, "zg": zg, "w1": w1, "w2": w2, "bias": bias})

    res = run_bass_kernel_spmd(nc, in_maps, core_ids=list(range(NCORES)))
    global LAST_RESULTS
    LAST_RESULTS = res

    out = np.empty((B, D), np.float32)
    for c in range(NCORES):
        outP = res.results[c]["outP"]
        xoff = 0
        for s, t0, tn in seq:
            k = assign[c][s]
            seg = idxs[k][t0:t0 + tn]
            n = len(seg)
            if n:
                blk = outP[:, xoff:xoff + KC1 * tn].reshape(128, KC1, tn)
                # blk[p, i, c] = out[token c, 128i + p]
                out[seg] = blk[:, :, :n].transpose(2, 1, 0).reshape(n, D)
            xoff += KC1 * tn
    return out


# revision 19
# speedup vs baseline: 1.4386x; 1.3494x over previous
"""Trainium2 Bass kernel for nn_DecSwitchedFC (MoE hard routing).

Math (per token b, expert e = y_idx[b]):
    out[b] = x[b] + z[b, e] * (relu(x[b] @ W1[e] + b1[e]) @ W2[e] + b2[e])

Strategy: expert-parallel over 8 NeuronCores, 2 experts per core.  The host
routes tokens to their experts (pure data movement — gather/scatter and
transpose), each core runs the two dense bottleneck FCs for its experts'
tokens on the tensor engine, and the host scatters rows back.  Only the
selected expert is computed per token (1/16 of the reference FLOPs).

Device data layout is fully "feature-major" (d or h on partitions, tokens on
the free axis) so no on-device transposes are needed:
    h^T[256, n]  = W1[e]^T(lhsT=W1) @ x^T          (K=1024, 8 chunks)
    o^T[1024, n] = W2[e]^T(lhsT=W2) @ relu(h^T+b1) (K=256, 2 chunks)

Modes (KMODE):
    f32  — exact fp32 matmuls (4 cyc/row), device applies z-scale+residual.
    f32r — fp32r matmuls (~1e-4 rel err, full rate), device z-scale+residual.
    bf16 — bf16 matmuls (~3e-3 rel err), o^T shipped back in bf16 and the
           z-scale + residual applied on the host in exact fp32 (halves DMA).

All host-side arrays are packed so every DMA is a single issue with 128
descriptors of contiguous bytes (one per SBUF partition) — DMA issue rate,
not bandwidth, limits the pipeline ramp otherwise.
"""

import os

import ml_dtypes
import numpy as np

import concourse.bacc as bacc
import concourse.mybir as mybir
import concourse.tile as tile
from concourse.bass_utils import run_bass_kernel_spmd

D = 1024        # model dim
H = 256         # bottleneck dim
NB = 16         # n experts
NCORES = 8
EPC = NB // NCORES   # experts per core
TILE_N = 512    # token-tile width (PSUM-bank / fp32 moving-operand max)
KC1 = D // 128  # contraction chunks for x @ W1
KC2 = H // 128  # contraction chunks for h @ W2
F32 = mybir.dt.float32
BF16 = mybir.dt.bfloat16

KMODE = os.environ.get("KMODE", "bf16")
MM_DT = {"f32": F32, "f32r": mybir.dt.float32r, "bf16": BF16}[KMODE]
NP_MM = ml_dtypes.bfloat16 if KMODE == "bf16" else np.float32
HOST_COMBINE = KMODE == "bf16"   # z-scale + residual on host
OUT_DT = BF16 if HOST_COMBINE else F32
NP_OUT = ml_dtypes.bfloat16 if HOST_COMBINE else np.float32

_build_cache: dict[tuple, object] = {}
LAST_RESULTS = None  # BassKernelResults of the most recent run (for profiling)


def _tile_seq(caps):
    """Global tile order: 128-wide tile first (fast PE start via a small
    first DMA) and 128-wide tile last (short epilogue/drain tail).
    Returns [(s, t0, tn), ...]."""
    def chunks(cap, lead128, tail128):
        widths = []
        rem = cap
        if lead128 and rem > 128:
            widths.append(128)
            rem -= 128
        tail = 0
        if tail128 and rem > 128:
            tail = 128
            rem -= 128
        while rem > 0:
            w = min(TILE_N, rem)
            widths.append(w)
            rem -= w
        if tail:
            widths.append(tail)
        out = []
        t0 = 0
        for w in widths:
            out.append((t0, w))
            t0 += w
        return out
    seq = [(0, t0, tn) for t0, tn in chunks(caps[0], True, False)]
    seq += [(1, t0, tn) for t0, tn in chunks(caps[1], False, True)]
    return seq


def _build(caps):
    key = (caps, KMODE)
    if key in _build_cache:
        return _build_cache[key]
    C = sum(caps)
    seq = _tile_seq(caps)
    xcols = KC1 * C   # total packed x / out columns

    nc = bacc.Bacc("TRN2", target_bir_lowering=False, debug=False)

    xg = nc.dram_tensor("xg", [128, xcols], MM_DT, kind="ExternalInput")
    w1 = nc.dram_tensor("w1", [128, EPC, KC1 * H], MM_DT, kind="ExternalInput")
    w2 = nc.dram_tensor("w2", [128, EPC, KC2 * D], MM_DT, kind="ExternalInput")
    # bias[p, e*10 + j]     = b1[e, 128j + p]   (j in 0..1)
    # bias[p, e*10 + 2 + i] = b2[e, 128i + p]   (i in 0..7)
    bias = nc.dram_tensor("bias", [128, EPC * (KC2 + KC1)], F32,
                          kind="ExternalInput")
    # packed out: per tile block of KC1*tn columns, [p, i*tn + c]
    outP = nc.dram_tensor("outP", [128, xcols], OUT_DT, kind="ExternalOutput")
    if not HOST_COMBINE:
        zg = nc.dram_tensor("zg", [128, C], F32, kind="ExternalInput")

    with tile.TileContext(nc) as tc:
        with (
            tc.tile_pool(name="const", bufs=1) as cpool,
            tc.tile_pool(name="w1p", bufs=2) as w1pool,
            tc.tile_pool(name="w2p", bufs=2) as w2pool,
            tc.tile_pool(name="xp", bufs=3) as xpool,
            tc.tile_pool(name="zp", bufs=3) as zpool,
            tc.tile_pool(name="hp", bufs=2) as hpool,
            tc.tile_pool(name="op", bufs=2) as opool,
            tc.tile_pool(name="ph", bufs=2, space="PSUM") as phpool,
            tc.tile_pool(name="po", bufs=4, space="PSUM") as popool,
        ):
            bias_t = None
            w1t = w2t = None
            cur_s = -1
            xoff = 0
            for q, (s, t0, tn) in enumerate(seq):
                new_s = s != cur_s
                cur_s = s
                if new_s:
                    w1t = w1pool.tile([128, KC1, H], MM_DT, tag="w1t")
                    nc.sync.dma_start(
                        w1t[:], w1[:, s].rearrange("p (k m) -> p k m", k=KC1))

                xt = xpool.tile([128, KC1, tn], MM_DT, tag="xt")
                nc.sync.dma_start(
                    xt[:],
                    xg[:, xoff:xoff + KC1 * tn].rearrange(
                        "p (k c) -> p k c", k=KC1))
                if not HOST_COMBINE:
                    c0 = (caps[0] if s else 0) + t0
                    zt = zpool.tile([128, tn], F32, tag="zt")
                    nc.sync.dma_start(zt[:], zg[:, c0:c0 + tn])
                if bias_t is None:
                    bias_t = cpool.tile([128, EPC * (KC2 + KC1)], F32)
                    nc.sync.dma_start(bias_t[:], bias[:])
                if new_s:
                    w2t = w2pool.tile([128, KC2, D], MM_DT, tag="w2t")
                    nc.sync.dma_start(
                        w2t[:], w2[:, s].rearrange("p (j m) -> p j m", j=KC2))

                ht = hpool.tile([128, KC2, tn], MM_DT, tag="ht")
                for j in range(KC2):
                    ph = phpool.tile([128, tn], F32, tag="ph")
                    for k in range(KC1):
                        nc.tensor.matmul(
                            ph[:], w1t[:, k, 128 * j:128 * (j + 1)],
                            xt[:, k, :],
                            start=(k == 0), stop=(k == KC1 - 1))
                    nc.scalar.activation(
                        ht[:, j, :], ph[:],
                        mybir.ActivationFunctionType.Relu,
                        bias=bias_t[:, s * 10 + j:s * 10 + j + 1])

                ot = opool.tile([128, KC1, tn], OUT_DT, tag="ot")
                last = q == len(seq) - 1
                for i in range(KC1):
                    po = popool.tile([128, tn], F32, tag="po")
                    for j in range(KC2):
                        nc.tensor.matmul(
                            po[:], w2t[:, j, 128 * i:128 * (i + 1)],
                            ht[:, j, :],
                            start=(j == 0), stop=(j == KC2 - 1))
                    bcol = bias_t[:, s * 10 + 2 + i:s * 10 + 3 + i]
                    if HOST_COMBINE:
                        # o + b2 (z-scale + residual happen on the host);
                        # alternate ACT/DVE so neither engine bottlenecks
                        if i % 2 == 0:
                            nc.scalar.activation(
                                ot[:, i, :], po[:],
                                mybir.ActivationFunctionType.Identity,
                                bias=bcol)
                        else:
                            nc.vector.tensor_scalar_add(ot[:, i, :], po[:],
                                                        bcol)
                    else:
                        # (o + b2) * z, then + x (residual)
                        nc.vector.scalar_tensor_tensor(
                            ot[:, i, :], po[:], bcol, zt[:],
                            mybir.AluOpType.add, mybir.AluOpType.mult)
                        nc.vector.tensor_add(ot[:, i, :], ot[:, i, :],
                                             xt[:, i, :].bitcast(F32))
                    if last:
                        nc.sync.dma_start(
                            outP[:, xoff + i * tn:xoff + (i + 1) * tn],
                            ot[:, i, :])
                if not last:
                    nc.sync.dma_start(
                        outP[:, xoff:xoff + KC1 * tn].rearrange(
                            "p (k c) -> p k c", k=KC1),
                        ot[:])

                xoff += KC1 * tn

    nc.compile()
    _build_cache[key] = nc
    return nc


def kernel(x, y_idx, y, z, W1, b1, W2, b2):
    x = np.ascontiguousarray(np.asarray(x, dtype=np.float32))
    z = np.asarray(z, dtype=np.float32)
    W1 = np.asarray(W1, dtype=np.float32)
    b1 = np.asarray(b1, dtype=np.float32)
    W2 = np.asarray(W2, dtype=np.float32)
    b2 = np.asarray(b2, dtype=np.float32)
    e = np.asarray(y_idx).reshape(-1).astype(np.int64)
    B = x.shape[0]

    idxs = [np.flatnonzero(e == k) for k in range(NB)]
    counts = np.array([len(i) for i in idxs])
    # Assign experts to (core, slot): top-8 counts in slot 0, bottom-8 in
    # slot 1, so the per-slot padded capacities are as small as possible.
    order = np.argsort(-counts, kind="stable")
    assign = [[int(order[c]), int(order[NB - 1 - c])] for c in range(NCORES)]

    def _cap(ns):
        return max(128, -(-max(ns) // 128) * 128)

    caps = (_cap([counts[assign[c][0]] for c in range(NCORES)]),
            _cap([counts[assign[c][1]] for c in range(NCORES)]))
    C = sum(caps)
    seq = _tile_seq(caps)
    xcols = KC1 * C

    nc = _build(caps)

    nbias = EPC * (KC2 + KC1)
    in_maps = []
    for c in range(NCORES):
        xg = np.zeros((128, xcols), NP_MM)
        bias = np.zeros((128, nbias), np.float32)
        w1 = np.empty((128, EPC, KC1 * H), NP_MM)
        w2 = np.empty((128, EPC, KC2 * D), NP_MM)
        if not HOST_COMBINE:
            zg = np.zeros((128, C), np.float32)
        for s in range(EPC):
            k = assign[c][s]
            idx = idxs[k]
            n = len(idx)
            if not HOST_COMBINE:
                c0 = caps[0] if s else 0
                zg[:, c0:c0 + n] = z[idx, k][None, :]
            bias[:, s * 10:s * 10 + KC2] = b1[k].reshape(KC2, 128).T
            bias[:, s * 10 + KC2:s * 10 + KC2 + KC1] = b2[k].reshape(KC1, 128).T
            w1[:, s] = W1[k].reshape(KC1, 128, H).transpose(1, 0, 2).reshape(
                128, KC1 * H).astype(NP_MM)
            w2[:, s] = W2[k].reshape(KC2, 128, D).transpose(1, 0, 2).reshape(
                128, KC2 * D).astype(NP_MM)
        xoff = 0
        for s, t0, tn in seq:
            k = assign[c][s]
            seg = idxs[k][t0:t0 + tn]
            n = len(seg)
            if n:
                full = np.zeros((128, KC1, tn), NP_MM)
                full[:, :, :n] = x[seg].reshape(
                    n, KC1, 128).transpose(2, 1, 0).astype(NP_MM)
                xg[:, xoff:xoff + KC1 * tn] = full.reshape(128, KC1 * tn)
            xoff += KC1 * tn
        m = {"xg": xg, "w1": w1, "w2": w2, "bias": bias}
        if not HOST_COMBINE:
            m["zg"] = zg
        in_maps.append(m)

    res = run_bass_kernel_spmd(nc, in_maps, core_ids=list(range(NCORES)))
    global LAST_RESULTS
    LAST_RESULTS = res

    out = np.empty((B, D), np.float32)
    for c in range(NCORES):
        outP = res.results[c]["outP"]
        xoff = 0
        for s, t0, tn in seq:
            k = assign[c][s]
            seg = idxs[k][t0:t0 + tn]
            n = len(seg)
            if n:
                blk = outP[:, xoff:xoff + KC1 * tn].reshape(128, KC1, tn)
                # blk[p, i, c] = o[token c, 128i + p]
                rows = blk[:, :, :n].transpose(2, 1, 0).reshape(
                    n, D).astype(np.float32)
                if HOST_COMBINE:
                    out[seg] = x[seg] + z[seg, k][:, None] * rows
                else:
                    out[seg] = rows
            xoff += KC1 * tn
    return out


# revision 20
# speedup vs baseline: 1.6118x; 1.1204x over previous
"""Trainium2 Bass kernel for nn_DecSwitchedFC (MoE hard routing).

Math (per token b, expert e = y_idx[b]):
    out[b] = x[b] + z[b, e] * (relu(x[b] @ W1[e] + b1[e]) @ W2[e] + b2[e])

Strategy: expert-parallel over 8 NeuronCores, 2 experts per core.  The host
routes tokens to their experts (pure data movement — gather/scatter and
transpose), each core runs the two dense bottleneck FCs for its experts'
tokens on the tensor engine, and the host scatters rows back.  Only the
selected expert is computed per token (1/16 of the reference FLOPs).

Device data layout is fully "feature-major" (d or h on partitions, tokens on
the free axis) so no on-device transposes are needed:
    h^T[256, n]  = W1[e]^T(lhsT=W1) @ x^T          (K=1024, 8 chunks)
    o^T[1024, n] = W2[e]^T(lhsT=W2) @ relu(h^T+b1) (K=256, 2 chunks)

Modes (KMODE):
    f32  — exact fp32 matmuls (4 cyc/row), device applies z-scale+residual.
    f32r — fp32r matmuls (~1e-4 rel err, full rate), device z-scale+residual.
    bf16 — bf16 matmuls (~3e-3 rel err), o^T shipped back in bf16 and the
           z-scale + residual applied on the host in exact fp32 (halves DMA).

All host-side arrays are packed so every DMA is a single issue with 128
descriptors of contiguous bytes (one per SBUF partition) — DMA issue rate,
not bandwidth, limits the pipeline ramp otherwise.
"""

import os

import ml_dtypes
import numpy as np

import concourse.bacc as bacc
import concourse.mybir as mybir
import concourse.tile as tile
from concourse.bass_utils import run_bass_kernel_spmd

D = 1024        # model dim
H = 256         # bottleneck dim
NB = 16         # n experts
NCORES = 8
EPC = NB // NCORES   # experts per core
TILE_N = 512    # token-tile width (PSUM-bank / fp32 moving-operand max)
KC1 = D // 128  # contraction chunks for x @ W1
KC2 = H // 128  # contraction chunks for h @ W2
F32 = mybir.dt.float32
BF16 = mybir.dt.bfloat16

KMODE = os.environ.get("KMODE", "bf16")
MM_DT = {"f32": F32, "f32r": mybir.dt.float32r, "bf16": BF16}[KMODE]
NP_MM = ml_dtypes.bfloat16 if KMODE == "bf16" else np.float32
HOST_COMBINE = KMODE == "bf16"   # z-scale + residual on host
OUT_DT = BF16 if HOST_COMBINE else F32
NP_OUT = ml_dtypes.bfloat16 if HOST_COMBINE else np.float32

_build_cache: dict[tuple, object] = {}
LAST_RESULTS = None  # BassKernelResults of the most recent run (for profiling)


def _tile_seq(caps):
    """Global tile order: 128-wide tile first (fast PE start via a small
    first DMA) and 128-wide tile last (short epilogue/drain tail).
    Returns [(s, t0, tn), ...]."""
    def chunks(cap, lead128, tail128):
        widths = []
        rem = cap
        if lead128 and rem > 128:
            widths.append(128)
            rem -= 128
        tail = 0
        if tail128 and rem > 128:
            tail = 128
            rem -= 128
        while rem > 0:
            w = min(TILE_N, rem)
            widths.append(w)
            rem -= w
        if tail:
            widths.append(tail)
        out = []
        t0 = 0
        for w in widths:
            out.append((t0, w))
            t0 += w
        return out
    seq = [(0, t0, tn) for t0, tn in chunks(caps[0], True, False)]
    seq += [(1, t0, tn) for t0, tn in chunks(caps[1], False, True)]
    return seq


def _build(caps):
    key = (caps, KMODE)
    if key in _build_cache:
        return _build_cache[key]
    C = sum(caps)
    seq = _tile_seq(caps)
    xcols = KC1 * C   # total packed x / out columns

    nc = bacc.Bacc("TRN2", target_bir_lowering=False, debug=False)

    xg = nc.dram_tensor("xg", [128, xcols], MM_DT, kind="ExternalInput")
    w1 = nc.dram_tensor("w1", [128, EPC, KC1 * H], MM_DT, kind="ExternalInput")
    w2 = nc.dram_tensor("w2", [128, EPC, KC2 * D], MM_DT, kind="ExternalInput")
    # bias[p, e*10 + j]     = b1[e, 128j + p]   (j in 0..1)
    # bias[p, e*10 + 2 + i] = b2[e, 128i + p]   (i in 0..7)
    bias = nc.dram_tensor("bias", [128, EPC * (KC2 + KC1)], F32,
                          kind="ExternalInput")
    # packed out: per tile block of KC1*tn columns, [p, i*tn + c]
    outP = nc.dram_tensor("outP", [128, xcols], OUT_DT, kind="ExternalOutput")
    if not HOST_COMBINE:
        zg = nc.dram_tensor("zg", [128, C], F32, kind="ExternalInput")

    with tile.TileContext(nc) as tc:
        with (
            tc.tile_pool(name="const", bufs=1) as cpool,
            tc.tile_pool(name="w1p", bufs=2) as w1pool,
            tc.tile_pool(name="w2p", bufs=2) as w2pool,
            tc.tile_pool(name="xp", bufs=3) as xpool,
            tc.tile_pool(name="zp", bufs=3) as zpool,
            tc.tile_pool(name="hp", bufs=2) as hpool,
            tc.tile_pool(name="op", bufs=2) as opool,
            tc.tile_pool(name="ph", bufs=3, space="PSUM") as phpool,
            tc.tile_pool(name="po", bufs=4, space="PSUM") as popool,
        ):
            bias_t = None
            w1t = w2t = None
            cur_s = -1
            xoff = 0
            for q, (s, t0, tn) in enumerate(seq):
                new_s = s != cur_s
                cur_s = s
                if new_s:
                    w1t = w1pool.tile([128, KC1, H], MM_DT, tag="w1t")
                    nc.sync.dma_start(
                        w1t[:], w1[:, s].rearrange("p (k m) -> p k m", k=KC1))

                xt = xpool.tile([128, KC1, tn], MM_DT, tag="xt")
                nc.sync.dma_start(
                    xt[:],
                    xg[:, xoff:xoff + KC1 * tn].rearrange(
                        "p (k c) -> p k c", k=KC1))
                if not HOST_COMBINE:
                    c0 = (caps[0] if s else 0) + t0
                    zt = zpool.tile([128, tn], F32, tag="zt")
                    nc.sync.dma_start(zt[:], zg[:, c0:c0 + tn])
                if bias_t is None:
                    bias_t = cpool.tile([128, EPC * (KC2 + KC1)], F32)
                    nc.sync.dma_start(bias_t[:], bias[:])
                if new_s:
                    w2t = w2pool.tile([128, KC2, D], MM_DT, tag="w2t")
                    nc.sync.dma_start(
                        w2t[:], w2[:, s].rearrange("p (j m) -> p j m", j=KC2))

                ht = hpool.tile([128, KC2, tn], MM_DT, tag="ht")
                for j in range(KC2):
                    ph = phpool.tile([128, tn], F32, tag="ph")
                    for k in range(KC1):
                        nc.tensor.matmul(
                            ph[:], w1t[:, k, 128 * j:128 * (j + 1)],
                            xt[:, k, :],
                            start=(k == 0), stop=(k == KC1 - 1))
                    nc.scalar.activation(
                        ht[:, j, :], ph[:],
                        mybir.ActivationFunctionType.Relu,
                        bias=bias_t[:, s * 10 + j:s * 10 + j + 1])

                ot = opool.tile([128, KC1, tn], OUT_DT, tag="ot")
                last = q == len(seq) - 1
                for i in range(KC1):
                    po = popool.tile([128, tn], F32, tag="po")
                    for j in range(KC2):
                        nc.tensor.matmul(
                            po[:], w2t[:, j, 128 * i:128 * (i + 1)],
                            ht[:, j, :],
                            start=(j == 0), stop=(j == KC2 - 1))
                    bcol = bias_t[:, s * 10 + 2 + i:s * 10 + 3 + i]
                    if HOST_COMBINE:
                        # o + b2 (z-scale + residual happen on the host);
                        # alternate ACT/DVE so neither engine bottlenecks
                        if i % 2 == 0:
                            nc.scalar.activation(
                                ot[:, i, :], po[:],
                                mybir.ActivationFunctionType.Identity,
                                bias=bcol)
                        else:
                            nc.vector.tensor_scalar_add(ot[:, i, :], po[:],
                                                        bcol)
                    else:
                        # (o + b2) * z, then + x (residual)
                        nc.vector.scalar_tensor_tensor(
                            ot[:, i, :], po[:], bcol, zt[:],
                            mybir.AluOpType.add, mybir.AluOpType.mult)
                        nc.vector.tensor_add(ot[:, i, :], ot[:, i, :],
                                             xt[:, i, :].bitcast(F32))
                    if last and i == KC1 // 2 - 1:
                        nc.sync.dma_start(
                            outP[:, xoff:xoff + (KC1 // 2) * tn].rearrange(
                                "p (k c) -> p k c", k=KC1 // 2),
                            ot[:, :KC1 // 2, :])
                if last:
                    nc.sync.dma_start(
                        outP[:, xoff + (KC1 // 2) * tn:
                             xoff + KC1 * tn].rearrange(
                            "p (k c) -> p k c", k=KC1 - KC1 // 2),
                        ot[:, KC1 // 2:, :])
                else:
                    nc.sync.dma_start(
                        outP[:, xoff:xoff + KC1 * tn].rearrange(
                            "p (k c) -> p k c", k=KC1),
                        ot[:])

                xoff += KC1 * tn

    nc.compile()
    _build_cache[key] = nc
    return nc


def kernel(x, y_idx, y, z, W1, b1, W2, b2):
    x = np.ascontiguousarray(np.asarray(x, dtype=np.float32))
    z = np.asarray(z, dtype=np.float32)
    W1 = np.asarray(W1, dtype=np.float32)
    b1 = np.asarray(b1, dtype=np.float32)
    W2 = np.asarray(W2, dtype=np.float32)
    b2 = np.asarray(b2, dtype=np.float32)
    e = np.asarray(y_idx).reshape(-1).astype(np.int64)
    B = x.shape[0]

    idxs = [np.flatnonzero(e == k) for k in range(NB)]
    counts = np.array([len(i) for i in idxs])
    # Assign experts to (core, slot): top-8 counts in slot 0, bottom-8 in
    # slot 1, so the per-slot padded capacities are as small as possible.
    order = np.argsort(-counts, kind="stable")
    assign = [[int(order[c]), int(order[NB - 1 - c])] for c in range(NCORES)]

    def _cap(ns):
        return max(128, -(-max(ns) // 128) * 128)

    caps = (_cap([counts[assign[c][0]] for c in range(NCORES)]),
            _cap([counts[assign[c][1]] for c in range(NCORES)]))
    C = sum(caps)
    seq = _tile_seq(caps)
    xcols = KC1 * C

    nc = _build(caps)

    nbias = EPC * (KC2 + KC1)
    in_maps = []
    for c in range(NCORES):
        xg = np.zeros((128, xcols), NP_MM)
        bias = np.zeros((128, nbias), np.float32)
        w1 = np.empty((128, EPC, KC1 * H), NP_MM)
        w2 = np.empty((128, EPC, KC2 * D), NP_MM)
        if not HOST_COMBINE:
            zg = np.zeros((128, C), np.float32)
        for s in range(EPC):
            k = assign[c][s]
            idx = idxs[k]
            n = len(idx)
            if not HOST_COMBINE:
                c0 = caps[0] if s else 0
                zg[:, c0:c0 + n] = z[idx, k][None, :]
            bias[:, s * 10:s * 10 + KC2] = b1[k].reshape(KC2, 128).T
            bias[:, s * 10 + KC2:s * 10 + KC2 + KC1] = b2[k].reshape(KC1, 128).T
            w1[:, s] = W1[k].reshape(KC1, 128, H).transpose(1, 0, 2).reshape(
                128, KC1 * H).astype(NP_MM)
            w2[:, s] = W2[k].reshape(KC2, 128, D).transpose(1, 0, 2).reshape(
                128, KC2 * D).astype(NP_MM)
        xoff = 0
        for s, t0, tn in seq:
            k = assign[c][s]
            seg = idxs[k][t0:t0 + tn]
            n = len(seg)
            if n:
                full = np.zeros((128, KC1, tn), NP_MM)
                full[:, :, :n] = x[seg].reshape(
                    n, KC1, 128).transpose(2, 1, 0).astype(NP_MM)
                xg[:, xoff:xoff + KC1 * tn] = full.reshape(128, KC1 * tn)
            xoff += KC1 * tn
        m = {"xg": xg, "w1": w1, "w2": w2, "bias": bias}
        if not HOST_COMBINE:
            m["zg"] = zg
        in_maps.append(m)

    res = run_bass_kernel_spmd(nc, in_maps, core_ids=list(range(NCORES)))
    global LAST_RESULTS
    LAST_RESULTS = res

    out = np.empty((B, D), np.float32)
    for c in range(NCORES):
        outP = res.results[c]["outP"]
        xoff = 0
        for s, t0, tn in seq:
            k = assign[c][s]
            seg = idxs[k][t0:t0 + tn]
            n = len(seg)
            if n:
                blk = outP[:, xoff:xoff + KC1 * tn].reshape(128, KC1, tn)
                # blk[p, i, c] = o[token c, 128i + p]
                rows = blk[:, :, :n].transpose(2, 1, 0).reshape(
                    n, D).astype(np.float32)
                if HOST_COMBINE:
                    out[seg] = x[seg] + z[seg, k][:, None] * rows
                else:
                    out[seg] = rows
            xoff += KC1 * tn
    return out
